# revision 2
# baseline (speedup 1.0000x reference)
"""Banded (sparse) attention + projections on 8 Trainium2 NeuronCores.

Problem: nn_Attention_old_90211493085279
  x [2, 2048, 1024] -> qkv = x @ Wqkv, banded softmax(QK^T) V (half-width 8),
  out = attn @ Wproj + bproj.

Sharding: (batch x tokens) across the 8 cores -- each core owns a contiguous
block of 512 token rows. The attention band is 17 wide, so each core needs an
8-token halo of K/V context only: NO collectives.

v2 changes vs the first working kernel (153 us):
  - inputs packed host-side into 7 large DMAs (xh, wv, 8x wqk blocks
    m-major, wp, masks) spread over the sync/scalar/gpsimd issue queues;
    the old version issued 91 descriptors at ~600 ns each on one queue and
    the first matmul started at 22.8 us.
  - PE warmup chain (dummy matmuls on zero tiles) so the HAM clock-gate is
    released (2.4 GHz) before real work arrives.
  - softmax normalization fully on-chip: DVE reciprocal on the PSUM
    denominator row, GpSimd partition_broadcast, fused multiply into otn.
    The old DRAM round-trips stalled the PE for ~15 us late in the kernel
    and re-throttled it to 1.2 GHz for the output projection.
  - single Tile region with hand-interleaved emission (engine queues are
    FIFO): per-fm qk-projection, paired-head banded attention, V-projection
    and output projection woven so the PE queue never waits.
"""

import sys

sys.path.insert(0, "/opt/trn_rl_repo")

import ml_dtypes
import numpy as np

import concourse.bass as bass
import concourse.tile as tile
from concourse import bacc, mybir
from concourse.bass_utils import run_bass_kernel_spmd

F32 = mybir.dt.float32
BF16 = mybir.dt.bfloat16
AF = mybir.ActivationFunctionType

B, N, C, H, HD, W = 2, 2048, 1024, 16, 64, 8
SCALE = float(HD) ** -0.5
CORES = 8
TOK = 512            # token rows owned per core
HALO = TOK + 2 * W   # 528 k/v context tokens per core
NT = TOK // 128      # 4 row tiles of 128
WIN = 128 + 2 * W    # 144 k/v window per row tile
NWARM = 80           # PE warmup matmuls (N=256 each)

_CACHE = {}


def _build_nc(dbg=False):
    nc = bacc.Bacc(None, target_bir_lowering=False)
    xhp = nc.dram_tensor("xhp", [128, 8 * HALO], BF16, kind="ExternalInput")
    wvp = nc.dram_tensor("wvp", [128, 8192], BF16, kind="ExternalInput")
    wqkp = nc.dram_tensor("wqkp", [128, 16384], BF16, kind="ExternalInput")
    wpp = nc.dram_tensor("wpp", [128, 8192], BF16, kind="ExternalInput")
    mA = nc.dram_tensor("mA", [128, TOK], BF16, kind="ExternalInput")
    mB = nc.dram_tensor("mB", [2 * W, TOK], BF16, kind="ExternalInput")
    bp = nc.dram_tensor("bp", [128, 8], F32, kind="ExternalInput")
    outT = nc.dram_tensor("outT", [C, TOK], F32, kind="ExternalOutput")

    vsizes = [128, 128, 128, 128, 2 * W]

    with tile.TileContext(nc) as tc:
        with tc.tile_pool(name="persist", bufs=1) as pp:
            # ---- persistent SBUF ----
            wmA = pp.tile([128, 128], BF16, tag="wmA", name="wmA")
            wmB = pp.tile([128, 256], BF16, tag="wmB", name="wmB")
            xh = pp.tile([128, 8 * HALO], BF16, tag="xh", name="xh")
            wv_sb = pp.tile([128, 8192], BF16, tag="wv", name="wv")
            wqk_sb = pp.tile([128, 16384], BF16, tag="wqk", name="wqk")
            wp_sb = pp.tile([128, 8192], BF16, tag="wp", name="wp")
            mask_a = pp.tile([128, TOK], BF16, tag="mask_a", name="mask_a")
            mask_b = pp.tile([2 * W, TOK], BF16, tag="mask_b", name="mask_b")
            bias_sb = pp.tile([128, 8], F32, tag="bias", name="bias")
            v1 = [pp.tile([p, H, HD + 1], BF16, tag=f"v1_{t}", name=f"v1_{t}")
                  for t, p in enumerate(vsizes)]
            qkT = [pp.tile([128, HALO], BF16, tag=f"qkT{m}", name=f"qkT{m}")
                   for m in range(16)]
            otn = [pp.tile([128, TOK], BF16, tag=f"otn{m}", name=f"otn{m}")
                   for m in range(8)]

            # warmup source tiles: first DVE work, no DMA dependency
            nc.vector.memset(wmA[:], 0.0)
            nc.vector.memset(wmB[:], 0.0)

            # ---- input DMAs: few, large, spread across issue queues ----
            # sync queue: xh first (everything needs it), then wv, then wp
            nc.sync.dma_start(out=xh[:], in_=xhp[:])
            nc.sync.dma_start(out=wv_sb[:, 0:4096], in_=wvp[:, 0:4096])
            nc.sync.dma_start(out=wv_sb[:, 4096:8192], in_=wvp[:, 4096:8192])
            nc.sync.dma_start(out=wp_sb[:], in_=wpp[:])
            # scalar queue: per-fm qk weight blocks (m-major packing)
            for fm in range(8):
                nc.scalar.dma_start(out=wqk_sb[:, 2048 * fm:2048 * (fm + 1)],
                                    in_=wqkp[:, 2048 * fm:2048 * (fm + 1)])
            # gpsimd (software DGE): small tensors
            nc.gpsimd.dma_start(out=mask_a[:], in_=mA[:])
            nc.gpsimd.dma_start(out=mask_b[:], in_=mB[:])
            nc.gpsimd.dma_start(out=bias_sb[:], in_=bp[:])

            # PSUM budget (8 banks of 2 KiB):
            #   pa 1 + pk 1 + sb(pb/stB shared) 1 + stA 1 + otb 2 + pv 2 = 8
            with tc.tile_pool(name="psQ", bufs=1, space="PSUM") as psQ, \
                 tc.tile_pool(name="psSB", bufs=1, space="PSUM") as psSB, \
                 tc.tile_pool(name="psSA", bufs=1, space="PSUM") as psSA, \
                 tc.tile_pool(name="psO", bufs=2, space="PSUM") as psO, \
                 tc.tile_pool(name="psV", bufs=2, space="PSUM") as psV, \
                 tc.tile_pool(name="atpa", bufs=2) as atpa, \
                 tc.tile_pool(name="atpb", bufs=2) as atpb, \
                 tc.tile_pool(name="recp", bufs=2) as recp, \
                 tc.tile_pool(name="bcp", bufs=2) as bcp, \
                 tc.tile_pool(name="outp", bufs=2) as outp:

                # ---- PE warmup: serialized dummy matmuls keep HAM busy
                # while the input DMAs stream ----
                psw = psV.tile([128, 512], F32, tag="pv", name="warm")
                for i in range(NWARM):
                    nc.tensor.matmul(psw[:, 0:256], wmA[:], wmB[:],
                                     start=True, stop=True)
                wsink = atpa.tile([128, TOK], BF16, tag="atA", name="wsink")
                nc.vector.tensor_copy(wsink[:, 0:256], psw[:, 0:256])

                ats = {}

                def emit_qkproj(fm):
                    # q chunk m=fm (own tokens), k chunk m=8+fm (full halo)
                    pa = psQ.tile([128, 512], F32, tag="pa", name="pa")
                    for c in range(8):
                        nc.tensor.matmul(
                            pa[:],
                            wqk_sb[:, 2048 * fm + 128 * c:2048 * fm + 128 * (c + 1)],
                            xh[:, 528 * c + W:528 * c + W + TOK],
                            start=(c == 0), stop=(c == 7))
                    nc.vector.tensor_copy(qkT[fm][:, W:W + TOK], pa[:])
                    pk = psQ.tile([128, 512], F32, tag="pk", name="pk")
                    for c in range(8):
                        nc.tensor.matmul(
                            pk[:],
                            wqk_sb[:, 2048 * fm + 1024 + 128 * c:
                                   2048 * fm + 1024 + 128 * (c + 1)],
                            xh[:, 528 * c:528 * c + 512],
                            start=(c == 0), stop=(c == 7))
                    nc.scalar.copy(qkT[8 + fm][:, 0:512], pk[:])
                    sb = psSB.tile([128, 512], F32, tag="sb", name="pb")
                    for c in range(8):
                        nc.tensor.matmul(
                            sb[:, 0:2 * W],
                            wqk_sb[:, 2048 * fm + 1024 + 128 * c:
                                   2048 * fm + 1024 + 128 * (c + 1)],
                            xh[:, 528 * c + 512:528 * c + 528],
                            start=(c == 0), stop=(c == 7))
                    nc.scalar.copy(qkT[8 + fm][:, 512:528], sb[:, 0:2 * W])

                def emit_scores(fm, h):
                    # transposed score strips + exp + band mask, one head
                    off = (h % 2) * 64
                    stA = psSA.tile([128, TOK], F32, tag="stA", name="stA")
                    sb = psSB.tile([128, 512], F32, tag="sb", name="stB")
                    for t in range(NT):
                        q_ap = qkT[fm][off:off + 64, W + 128 * t:W + 128 * t + 128]
                        k1 = qkT[8 + fm][off:off + 64, 128 * t:128 * t + 128]
                        nc.tensor.matmul(stA[:, 128 * t:128 * (t + 1)], k1, q_ap,
                                         start=True, stop=True)
                    for t in range(NT):
                        q_ap = qkT[fm][off:off + 64, W + 128 * t:W + 128 * t + 128]
                        k2 = qkT[8 + fm][off:off + 64, 128 * t + 128:128 * t + WIN]
                        nc.tensor.matmul(sb[0:2 * W, 128 * t:128 * (t + 1)], k2,
                                         q_ap, start=True, stop=True)
                    atA = atpa.tile([128, TOK], BF16, tag="atA", name="atA")
                    atB = atpb.tile([2 * W, TOK], BF16, tag="atB", name="atB")
                    nc.scalar.activation(atA[:], stA[:], AF.Exp)
                    nc.scalar.activation(atB[:], sb[0:2 * W, :], AF.Exp)
                    nc.vector.tensor_mul(atA[:], atA[:], mask_a[:])
                    nc.vector.tensor_mul(atB[:], atB[:], mask_b[:])
                    ats[h] = (atA, atB)

                def emit_av(fm, h):
                    # O^T strip [65, 512] (65th row = softmax denominator),
                    # then on-chip normalization into otn
                    off = (h % 2) * 64
                    atA, atB = ats.pop(h)
                    otb = psO.tile([128, TOK], F32, tag="otb", name="otb")
                    for t in range(NT):
                        nc.tensor.matmul(otb[0:HD + 1, 128 * t:128 * (t + 1)],
                                         v1[t][:, h, :],
                                         atA[:, 128 * t:128 * (t + 1)],
                                         start=True, stop=False)
                        nc.tensor.matmul(otb[0:HD + 1, 128 * t:128 * (t + 1)],
                                         v1[t + 1][0:2 * W, h, :],
                                         atB[:, 128 * t:128 * (t + 1)],
                                         start=False, stop=True)
                    rec = recp.tile([1, TOK], F32, tag="rec", name="rec")
                    nc.vector.reciprocal(rec[:], otb[HD:HD + 1, :])
                    bc = bcp.tile([HD, TOK], F32, tag="bc", name="bc")
                    nc.gpsimd.partition_broadcast(bc[:], rec[0:1, :])
                    nc.vector.tensor_mul(otn[fm][off:off + 64, :],
                                         otb[0:HD, :], bc[:])

                def emit_vproj():
                    # v = x @ Wv in natural [token, head, dim+1] layout,
                    # 65th column = 1.0 so AV also sums the denominators
                    for t in range(5):
                        p = vsizes[t]
                        pv0 = psV.tile([128, 512], F32, tag="pv", name="pv0")
                        pv1 = psV.tile([128, 512], F32, tag="pv", name="pv1")
                        for c in range(8):
                            nc.tensor.matmul(
                                pv0[:p, :],
                                xh[:, 528 * c + 128 * t:528 * c + 128 * t + p],
                                wv_sb[:, 512 * c:512 * c + 512],
                                start=(c == 0), stop=(c == 7))
                        for c in range(8):
                            nc.tensor.matmul(
                                pv1[:p, :],
                                xh[:, 528 * c + 128 * t:528 * c + 128 * t + p],
                                wv_sb[:, 4096 + 512 * c:4096 + 512 * c + 512],
                                start=(c == 0), stop=(c == 7))
                        nc.vector.tensor_copy(
                            v1[t][:, 0:8, 0:HD],
                            pv0[:p, :].rearrange("p (h d) -> p h d", d=HD))
                        nc.vector.tensor_copy(
                            v1[t][:, 8:16, 0:HD],
                            pv1[:p, :].rearrange("p (h d) -> p h d", d=HD))
                        nc.vector.memset(v1[t][:, :, HD], 1.0)

                # ---- emission order (engine queues are FIFO) ----
                emit_qkproj(0)
                emit_scores(0, 0)
                emit_scores(0, 1)
                emit_qkproj(1)
                emit_vproj()
                emit_scores(1, 2)
                emit_av(0, 0)
                emit_scores(1, 3)
                emit_av(0, 1)
                for fm in range(2, 8):
                    emit_qkproj(fm)
                    emit_av(fm - 1, 2 * fm - 2)
                    emit_scores(fm, 2 * fm)
                    emit_av(fm - 1, 2 * fm - 1)
                    emit_scores(fm, 2 * fm + 1)
                emit_av(7, 14)
                emit_av(7, 15)

                # ---- output projection (transposed) + bias ----
                for m in range(8):
                    pf = psV.tile([128, 512], F32, tag="pv", name="pf")
                    for c in range(8):
                        nc.tensor.matmul(
                            pf[:],
                            wp_sb[:, 1024 * c + 128 * m:1024 * c + 128 * (m + 1)],
                            otn[c][:],
                            start=(c == 0), stop=(c == 7))
                    ob = outp.tile([128, 512], F32, tag="ob", name="ob")
                    nc.vector.tensor_scalar_add(ob[:], pf[:], bias_sb[:, m:m + 1])
                    nc.sync.dma_start(out=outT[128 * m:128 * (m + 1), :], in_=ob[:])

    nc.finalize()
    return nc


def _get_nc(dbg=False):
    key = ("nc", dbg)
    if key not in _CACHE:
        _CACHE[key] = _build_nc(dbg)
    return _CACHE[key]


def _band_mask_np(n, w):
    i = np.arange(n)[:, None]
    j = np.arange(n)[None, :]
    lo = np.where(i <= w, 0, i - w)
    hi = np.where(n - i <= w, n - 1, i + w)
    return (j >= lo) & (j <= hi)


def _make_in_maps(x, Wqkv, Wproj, bproj):
    x = np.ascontiguousarray(np.asarray(x, dtype=np.float32))
    Wqkv = np.asarray(Wqkv, dtype=np.float32)
    Wproj = np.ascontiguousarray(np.asarray(Wproj, dtype=np.float32))
    bproj = np.asarray(bproj, dtype=np.float32)

    wqk_host = np.concatenate(
        [Wqkv[:, :C] * np.float32(SCALE), Wqkv[:, C:2 * C]], axis=1)
    wqk_host = np.ascontiguousarray(wqk_host).astype(ml_dtypes.bfloat16)
    wv_host = np.ascontiguousarray(Wqkv[:, 2 * C:]).astype(ml_dtypes.bfloat16)
    wp_host = Wproj.astype(ml_dtypes.bfloat16)
    bp_host = np.ascontiguousarray(bproj.reshape(8, 128).T)
    band = _band_mask_np(N, W)

    # packed weight layouts (shared by all cores)
    wqkp_host = np.concatenate(
        [np.concatenate(
            [wqk_host[128 * c:128 * (c + 1), 128 * fm:128 * (fm + 1)]
             for c in range(8)] +
            [wqk_host[128 * c:128 * (c + 1), 1024 + 128 * fm:1024 + 128 * (fm + 1)]
             for c in range(8)], axis=1)
         for fm in range(8)], axis=1)
    wqkp_host = np.ascontiguousarray(wqkp_host)
    wvp_host = np.concatenate(
        [np.concatenate([wv_host[128 * c:128 * (c + 1), 512 * half:512 * (half + 1)]
                         for c in range(8)], axis=1)
         for half in range(2)], axis=1)
    wvp_host = np.ascontiguousarray(wvp_host)
    wpp_host = np.concatenate(
        [wp_host[128 * c:128 * (c + 1), :] for c in range(8)], axis=1)
    wpp_host = np.ascontiguousarray(wpp_host)

    in_maps = []
    for core in range(CORES):
        b, qt = divmod(core, NT)
        g0 = qt * TOK
        xhrows = np.zeros((HALO, C), np.float32)
        s = max(0, g0 - W)
        e = min(N, g0 + TOK + W)
        xhrows[s - (g0 - W):e - (g0 - W)] = x[b, s:e]
        xhT_host = np.ascontiguousarray(xhrows.T).astype(ml_dtypes.bfloat16)
        xhp_host = np.ascontiguousarray(np.concatenate(
            [xhT_host[128 * c:128 * (c + 1), :] for c in range(8)], axis=1))

        mAh = np.zeros((128, TOK), np.float32)
        mBh = np.zeros((2 * W, TOK), np.float32)
        for t in range(NT):
            i = g0 + 128 * t + np.arange(128)[None, :]
            jw = (g0 - W) + 128 * t + np.arange(WIN)[:, None]
            valid = (jw >= 0) & (jw < N)
            mm = band[i, np.clip(jw, 0, N - 1)] & valid
            mAh[:, 128 * t:128 * (t + 1)] = mm[:128]
            mBh[:, 128 * t:128 * (t + 1)] = mm[128:]
        in_maps.append({
            "xhp": xhp_host, "wvp": wvp_host, "wqkp": wqkp_host,
            "wpp": wpp_host, "bp": bp_host,
            "mA": mAh.astype(ml_dtypes.bfloat16),
            "mB": mBh.astype(ml_dtypes.bfloat16),
        })
    return in_maps


def run_spmd(x, Wqkv, Wproj, bproj, dbg=False, **kw):
    """Run the SPMD kernel; returns (output, BassKernelResults)."""
    nc = _get_nc(dbg)
    in_maps = _make_in_maps(x, Wqkv, Wproj, bproj)
    res = run_bass_kernel_spmd(nc, in_maps, list(range(CORES)), **kw)
    outT = np.concatenate([res.results[i]["outT"] for i in range(CORES)], axis=1)
    out = np.ascontiguousarray(outT.T).reshape(B, N, C)
    return out, res


def kernel(x, Wqkv, Wproj, bproj):
    out, _ = run_spmd(x, Wqkv, Wproj, bproj)
    return out


# revision 8
# speedup vs baseline: 1.4478x; 1.4478x over previous
"""Banded (sparse) attention + projections on 8 Trainium2 NeuronCores.

Problem: nn_Attention_old_90211493085279
  x [2, 2048, 1024] -> qkv = x @ Wqkv, banded softmax(QK^T) V (half-width 8),
  out = attn @ Wproj + bproj.

Sharding: (batch x tokens) across the 8 cores -- each core owns a contiguous
block of 512 token rows. The attention band is 17 wide, so each core needs an
8-token halo of K/V context only: NO collectives.

v3 design:
  - inputs packed host-side into a few large DMAs spread over the
    sync/scalar/gpsimd issue queues (a dma_start costs ~600 ns of issue time
    on its queue engine); xh + the first qk weight block go first so real
    matmuls can start ~14 us in.
  - PE warmup chain (dummy matmuls on zero tiles) so the HAM clock-gate is
    released (2.4 GHz) before real work arrives.
  - AV matmul runs in NATURAL layout (attention-weight strip is the
    stationary operand, V the moving one), so the softmax denominator lands
    per-partition: normalization is a native reciprocal [128,4] +
    tensor_scalar_mul -- no partition broadcast, no DRAM round-trip. A [1,512]
    single-partition reciprocal costs 4 us on DVE; this costs ~0.1 us.
  - normalized O tiles are transposed to the [dims, tokens] layout the output
    projection needs via XBAR DMA-transposes on the otherwise idle DMA
    engines.
  - output projection is c-outer across 8 PSUM banks (reusing every
    attention pool's bank), so only the last 8-matmul column depends on the
    final head; bias is folded in as a K=1 matmul against a ones row.
"""

import sys

sys.path.insert(0, "/opt/trn_rl_repo")

import ml_dtypes
import numpy as np

import concourse.bass as bass
import concourse.tile as tile
from concourse import bacc, mybir
from concourse.bass_utils import run_bass_kernel_spmd

F32 = mybir.dt.float32
BF16 = mybir.dt.bfloat16
AF = mybir.ActivationFunctionType

B, N, C, H, HD, W = 2, 2048, 1024, 16, 64, 8
SCALE = float(HD) ** -0.5
CORES = 8
TOK = 512            # token rows owned per core
HALO = TOK + 2 * W   # 528 k/v context tokens per core
NT = TOK // 128      # 4 row tiles of 128
WIN = 128 + 2 * W    # 144 k/v window per row tile
NWARM = 48           # PE warmup matmuls (N=256 each)

_CACHE = {}


def _build_nc(dbg=False):
    nc = bacc.Bacc(None, target_bir_lowering=False)
    xhp = nc.dram_tensor("xhp", [128, 8 * HALO], BF16, kind="ExternalInput")
    wvp = nc.dram_tensor("wvp", [128, 8192], BF16, kind="ExternalInput")
    wqkp = nc.dram_tensor("wqkp", [128, 16384], BF16, kind="ExternalInput")
    wpp = nc.dram_tensor("wpp", [128, 8192], BF16, kind="ExternalInput")
    mA = nc.dram_tensor("mA", [128, TOK], BF16, kind="ExternalInput")
    mB = nc.dram_tensor("mB", [2 * W, TOK], BF16, kind="ExternalInput")
    bT = nc.dram_tensor("bT", [1, C], BF16, kind="ExternalInput")
    outT = nc.dram_tensor("outT", [C, TOK], F32, kind="ExternalOutput")

    vsizes = [128, 128, 128, 128, 2 * W]

    with tile.TileContext(nc) as tc:
        with tc.tile_pool(name="persist", bufs=1) as pp:
            # ---- persistent SBUF ----
            wmA = pp.tile([128, 128], BF16, tag="wmA", name="wmA")
            wmB = pp.tile([128, 256], BF16, tag="wmB", name="wmB")
            ones_r = pp.tile([1, TOK], BF16, tag="ones_r", name="ones_r")
            xh = pp.tile([128, 8 * HALO], BF16, tag="xh", name="xh")
            wv_sb = pp.tile([128, 8192], BF16, tag="wv", name="wv")
            wqk_sb = pp.tile([128, 16384], BF16, tag="wqk", name="wqk")
            wp_sb = pp.tile([128, 8192], BF16, tag="wp", name="wp")
            mask_a = pp.tile([128, TOK], BF16, tag="mask_a", name="mask_a")
            mask_b = pp.tile([2 * W, TOK], BF16, tag="mask_b", name="mask_b")
            biasT = pp.tile([1, C], BF16, tag="biasT", name="biasT")
            v1 = [pp.tile([p, H, HD + 1], BF16, tag=f"v1_{t}", name=f"v1_{t}")
                  for t, p in enumerate(vsizes)]
            qkT = [pp.tile([128, HALO], BF16, tag=f"qkT{m}", name=f"qkT{m}")
                   for m in range(16)]
            otn = [pp.tile([128, TOK], BF16, tag=f"otn{m}", name=f"otn{m}")
                   for m in range(8)]

            # warmup sources: first DVE work, no DMA dependency
            nc.vector.memset(wmA[:], 0.0)
            nc.vector.memset(wmB[:], 0.0)
            nc.vector.memset(ones_r[:], 1.0)
            for t in range(5):
                # ones column of v1 (disjoint from the v-proj copy columns)
                nc.vector.memset(v1[t][:, :, HD], 1.0)

            # ---- input DMAs: few, large, spread across issue queues ----
            # sync queue: the critical pair first (xh + first qk block)
            nc.sync.dma_start(out=xh[:], in_=xhp[:])
            nc.sync.dma_start(out=wqk_sb[:, 0:2048], in_=wqkp[:, 0:2048])
            nc.sync.dma_start(out=wv_sb[:, 0:4096], in_=wvp[:, 0:4096])
            nc.sync.dma_start(out=wv_sb[:, 4096:8192], in_=wvp[:, 4096:8192])
            nc.sync.dma_start(out=wp_sb[:], in_=wpp[:])
            # scalar queue: remaining qk weight blocks (m-major packing)
            for fm in range(1, 8):
                nc.scalar.dma_start(out=wqk_sb[:, 2048 * fm:2048 * (fm + 1)],
                                    in_=wqkp[:, 2048 * fm:2048 * (fm + 1)])
            # gpsimd (software DGE): small tensors
            nc.gpsimd.dma_start(out=mask_a[:], in_=mA[:])
            nc.gpsimd.dma_start(out=mask_b[:], in_=mB[:])
            nc.gpsimd.dma_start(out=biasT[:], in_=bT[:])

            # PSUM budget (8 banks of 2 KiB):
            #   pa 1 + pk 1 + sb(pb/stB shared) 1 + stA 1 + otb 2 + pv 2 = 8
            with tc.tile_pool(name="psQ", bufs=1, space="PSUM") as psQ, \
                 tc.tile_pool(name="psSB", bufs=1, space="PSUM") as psSB, \
                 tc.tile_pool(name="psSA", bufs=1, space="PSUM") as psSA, \
                 tc.tile_pool(name="psO", bufs=2, space="PSUM") as psO, \
                 tc.tile_pool(name="psV", bufs=2, space="PSUM") as psV, \
                 tc.tile_pool(name="atpa", bufs=2) as atpa, \
                 tc.tile_pool(name="atpb", bufs=2) as atpb, \
                 tc.tile_pool(name="recp", bufs=2) as recp, \
                 tc.tile_pool(name="onp", bufs=2) as onp, \
                 tc.tile_pool(name="outp", bufs=4) as outp:

                # ---- PE warmup: serialized dummy matmuls keep HAM busy
                # while the input DMAs stream ----
                psw = psV.tile([128, 512], F32, tag="pv", name="warm")
                for i in range(NWARM):
                    nc.tensor.matmul(psw[:, 0:256], wmA[:], wmB[:],
                                     start=True, stop=True)
                wsink = atpa.tile([128, TOK], BF16, tag="atA", name="wsink")
                nc.vector.tensor_copy(wsink[:, 0:256], psw[:, 0:256])

                ats = {}
                onats = {}

                def emit_qkproj(fm):
                    # q chunk m=fm (own tokens), k chunk m=8+fm (full halo)
                    pa = psQ.tile([128, 512], F32, tag="pa", name="pa")
                    for c in range(8):
                        nc.tensor.matmul(
                            pa[:],
                            wqk_sb[:, 2048 * fm + 128 * c:2048 * fm + 128 * (c + 1)],
                            xh[:, 528 * c + W:528 * c + W + TOK],
                            start=(c == 0), stop=(c == 7))
                    nc.vector.tensor_copy(qkT[fm][:, W:W + TOK], pa[:])
                    pk = psQ.tile([128, 512], F32, tag="pk", name="pk")
                    for c in range(8):
                        nc.tensor.matmul(
                            pk[:],
                            wqk_sb[:, 2048 * fm + 1024 + 128 * c:
                                   2048 * fm + 1024 + 128 * (c + 1)],
                            xh[:, 528 * c:528 * c + 512],
                            start=(c == 0), stop=(c == 7))
                    nc.scalar.copy(qkT[8 + fm][:, 0:512], pk[:])
                    sb = psSB.tile([128, 512], F32, tag="sb", name="pb")
                    for c in range(8):
                        nc.tensor.matmul(
                            sb[:, 0:2 * W],
                            wqk_sb[:, 2048 * fm + 1024 + 128 * c:
                                   2048 * fm + 1024 + 128 * (c + 1)],
                            xh[:, 528 * c + 512:528 * c + 528],
                            start=(c == 0), stop=(c == 7))
                    nc.scalar.copy(qkT[8 + fm][:, 512:528], sb[:, 0:2 * W])

                def emit_scores(fm, h):
                    # transposed score strips + exp + band mask, one head
                    off = (h % 2) * 64
                    stA = psSA.tile([128, TOK], F32, tag="stA", name="stA")
                    sb = psSB.tile([128, 512], F32, tag="sb", name="stB")
                    for t in range(NT):
                        q_ap = qkT[fm][off:off + 64, W + 128 * t:W + 128 * t + 128]
                        k1 = qkT[8 + fm][off:off + 64, 128 * t:128 * t + 128]
                        nc.tensor.matmul(stA[:, 128 * t:128 * (t + 1)], k1, q_ap,
                                         start=True, stop=True)
                    for t in range(NT):
                        q_ap = qkT[fm][off:off + 64, W + 128 * t:W + 128 * t + 128]
                        k2 = qkT[8 + fm][off:off + 64, 128 * t + 128:128 * t + WIN]
                        nc.tensor.matmul(sb[0:2 * W, 128 * t:128 * (t + 1)], k2,
                                         q_ap, start=True, stop=True)
                    atA = atpa.tile([128, TOK], BF16, tag="atA", name="atA")
                    atB = atpb.tile([2 * W, TOK], BF16, tag="atB", name="atB")
                    nc.scalar.activation(atA[:], stA[:], AF.Exp)
                    nc.scalar.activation(atB[:], sb[0:2 * W, :], AF.Exp)
                    nc.vector.tensor_mul(atA[:], atA[:], mask_a[:])
                    nc.vector.tensor_mul(atB[:], atB[:], mask_b[:])
                    ats[h] = (atA, atB)

                def emit_av(fm, h):
                    # natural-layout O strip per tile: [128 q, 65] where
                    # col 64 = softmax denominator (per-partition!)
                    off = (h % 2) * 64
                    atA, atB = ats.pop(h)
                    onat = psO.tile([128, 512], F32, tag="otb", name="onat")
                    for t in range(NT):
                        nc.tensor.matmul(onat[:, 65 * t:65 * t + 65],
                                         atA[:, 128 * t:128 * (t + 1)],
                                         v1[t][:, h, :],
                                         start=True, stop=False)
                        nc.tensor.matmul(onat[:, 65 * t:65 * t + 65],
                                         atB[:, 128 * t:128 * (t + 1)],
                                         v1[t + 1][0:2 * W, h, :],
                                         start=False, stop=True)
                    if fm not in onats:
                        onats[fm] = onp.tile([128, NT, 128], BF16, tag="on",
                                             name="on")
                    on_nat = onats[fm]
                    rec = recp.tile([128, NT], F32, tag="rec", name="rec")
                    nc.vector.reciprocal(
                        rec[:],
                        onat[:, 0:260].rearrange("p (t x) -> p t x", x=65)
                        [:, :, 64])
                    for t in range(NT):
                        nc.vector.tensor_scalar_mul(
                            on_nat[:, t, off:off + 64],
                            onat[:, 65 * t:65 * t + 64],
                            rec[:, t:t + 1])

                def emit_transposes(fm):
                    # [q, dims] -> [dims, q] via XBAR DMA (idle DMA engines)
                    on_nat = onats.pop(fm)
                    for t in range(NT):
                        nc.sync.dma_start(out=otn[fm][:, 128 * t:128 * (t + 1)],
                                          in_=on_nat[:, t, :], transpose=True)

                def emit_vproj():
                    # v = x @ Wv in natural [token, head, dim+1] layout,
                    # 65th column = 1.0 so AV also sums the denominators.
                    # half0 groups first (their weights arrive first).
                    for half in range(2):
                        for t in range(5):
                            p = vsizes[t]
                            pv = psV.tile([128, 512], F32, tag="pv",
                                          name=f"pv{half}_{t}")
                            for c in range(8):
                                nc.tensor.matmul(
                                    pv[:p, :],
                                    xh[:, 528 * c + 128 * t:528 * c + 128 * t + p],
                                    wv_sb[:, 4096 * half + 512 * c:
                                          4096 * half + 512 * c + 512],
                                    start=(c == 0), stop=(c == 7))
                            nc.vector.tensor_copy(
                                v1[t][:, 8 * half:8 * half + 8, 0:HD],
                                pv[:p, :].rearrange("p (h d) -> p h d", d=HD))

                # ---- emission order (engine queues are FIFO) ----
                emit_qkproj(0)
                emit_scores(0, 0)
                emit_scores(0, 1)
                emit_qkproj(1)
                emit_vproj()
                emit_scores(1, 2)
                emit_av(0, 0)
                emit_scores(1, 3)
                emit_av(0, 1)
                emit_transposes(0)
                for fm in range(2, 8):
                    emit_qkproj(fm)
                    emit_av(fm - 1, 2 * fm - 2)
                    emit_scores(fm, 2 * fm)
                    emit_av(fm - 1, 2 * fm - 1)
                    emit_transposes(fm - 1)
                    emit_scores(fm, 2 * fm + 1)
                emit_av(7, 14)
                emit_av(7, 15)
                emit_transposes(7)

                # ---- output projection, c-outer over 8 PSUM banks ----
                pf = [psV.tile([128, 512], F32, tag="pv", name="pf0"),
                      psV.tile([128, 512], F32, tag="pv", name="pf1"),
                      psQ.tile([128, 512], F32, tag="pa", name="pf2"),
                      psQ.tile([128, 512], F32, tag="pk", name="pf3"),
                      psSA.tile([128, 512], F32, tag="stA", name="pf4"),
                      psSB.tile([128, 512], F32, tag="sb", name="pf5"),
                      psO.tile([128, 512], F32, tag="otb", name="pf6"),
                      psO.tile([128, 512], F32, tag="otb", name="pf7")]
                for c in range(8):
                    for m in range(8):
                        nc.tensor.matmul(
                            pf[m][:],
                            wp_sb[:, 1024 * c + 128 * m:1024 * c + 128 * (m + 1)],
                            otn[c][:],
                            start=(c == 0), stop=False)
                for m in range(8):
                    # bias folded in as a K=1 matmul against a ones row
                    nc.tensor.matmul(pf[m][:], biasT[0:1, 128 * m:128 * (m + 1)],
                                     ones_r[0:1, :], start=False, stop=True)
                for m in range(8):
                    ob = outp.tile([128, 512], F32, tag="ob", name="ob")
                    if m % 2 == 0:
                        nc.scalar.copy(ob[:], pf[m][:])
                    else:
                        nc.vector.tensor_copy(ob[:], pf[m][:])
                    nc.sync.dma_start(out=outT[128 * m:128 * (m + 1), :],
                                      in_=ob[:])

    nc.finalize()
    return nc


def _get_nc(dbg=False):
    key = ("nc", dbg)
    if key not in _CACHE:
        _CACHE[key] = _build_nc(dbg)
    return _CACHE[key]


def _band_mask_np(n, w):
    i = np.arange(n)[:, None]
    j = np.arange(n)[None, :]
    lo = np.where(i <= w, 0, i - w)
    hi = np.where(n - i <= w, n - 1, i + w)
    return (j >= lo) & (j <= hi)


def _make_in_maps(x, Wqkv, Wproj, bproj):
    x = np.ascontiguousarray(np.asarray(x, dtype=np.float32))
    Wqkv = np.asarray(Wqkv, dtype=np.float32)
    Wproj = np.ascontiguousarray(np.asarray(Wproj, dtype=np.float32))
    bproj = np.asarray(bproj, dtype=np.float32)

    wqk_host = np.concatenate(
        [Wqkv[:, :C] * np.float32(SCALE), Wqkv[:, C:2 * C]], axis=1)
    wqk_host = np.ascontiguousarray(wqk_host).astype(ml_dtypes.bfloat16)
    wv_host = np.ascontiguousarray(Wqkv[:, 2 * C:]).astype(ml_dtypes.bfloat16)
    wp_host = Wproj.astype(ml_dtypes.bfloat16)
    bp_host = np.ascontiguousarray(bproj.reshape(8, 128).T)
    bT_host = np.ascontiguousarray(bproj.reshape(1, C)).astype(ml_dtypes.bfloat16)
    band = _band_mask_np(N, W)

    # packed weight layouts (shared by all cores)
    wqkp_host = np.concatenate(
        [np.concatenate(
            [wqk_host[128 * c:128 * (c + 1), 128 * fm:128 * (fm + 1)]
             for c in range(8)] +
            [wqk_host[128 * c:128 * (c + 1), 1024 + 128 * fm:1024 + 128 * (fm + 1)]
             for c in range(8)], axis=1)
         for fm in range(8)], axis=1)
    wqkp_host = np.ascontiguousarray(wqkp_host)
    wvp_host = np.concatenate(
        [np.concatenate([wv_host[128 * c:128 * (c + 1), 512 * half:512 * (half + 1)]
                         for c in range(8)], axis=1)
         for half in range(2)], axis=1)
    wvp_host = np.ascontiguousarray(wvp_host)
    wpp_host = np.concatenate(
        [wp_host[128 * c:128 * (c + 1), :] for c in range(8)], axis=1)
    wpp_host = np.ascontiguousarray(wpp_host)

    in_maps = []
    for core in range(CORES):
        b, qt = divmod(core, NT)
        g0 = qt * TOK
        xhrows = np.zeros((HALO, C), np.float32)
        s = max(0, g0 - W)
        e = min(N, g0 + TOK + W)
        xhrows[s - (g0 - W):e - (g0 - W)] = x[b, s:e]
        xhT_host = np.ascontiguousarray(xhrows.T).astype(ml_dtypes.bfloat16)
        xhp_host = np.ascontiguousarray(np.concatenate(
            [xhT_host[128 * c:128 * (c + 1), :] for c in range(8)], axis=1))

        mAh = np.zeros((128, TOK), np.float32)
        mBh = np.zeros((2 * W, TOK), np.float32)
        for t in range(NT):
            i = g0 + 128 * t + np.arange(128)[None, :]
            jw = (g0 - W) + 128 * t + np.arange(WIN)[:, None]
            valid = (jw >= 0) & (jw < N)
            mm = band[i, np.clip(jw, 0, N - 1)] & valid
            mAh[:, 128 * t:128 * (t + 1)] = mm[:128]
            mBh[:, 128 * t:128 * (t + 1)] = mm[128:]
        in_maps.append({
            "xhp": xhp_host, "wvp": wvp_host, "wqkp": wqkp_host,
            "wpp": wpp_host, "bT": bT_host,
            "mA": mAh.astype(ml_dtypes.bfloat16),
            "mB": mBh.astype(ml_dtypes.bfloat16),
        })
    return in_maps


def run_spmd(x, Wqkv, Wproj, bproj, dbg=False, **kw):
    """Run the SPMD kernel; returns (output, BassKernelResults)."""
    nc = _get_nc(dbg)
    in_maps = _make_in_maps(x, Wqkv, Wproj, bproj)
    res = run_bass_kernel_spmd(nc, in_maps, list(range(CORES)), **kw)
    outT = np.concatenate([res.results[i]["outT"] for i in range(CORES)], axis=1)
    out = np.ascontiguousarray(outT.T).reshape(B, N, C)
    return out, res


def kernel(x, Wqkv, Wproj, bproj):
    out, _ = run_spmd(x, Wqkv, Wproj, bproj)
    return out


# revision 15
# speedup vs baseline: 1.7538x; 1.2114x over previous
"""Banded (sparse) attention + projections on 8 Trainium2 NeuronCores.

Problem: nn_Attention_old_90211493085279
  x [2, 2048, 1024] -> qkv = x @ Wqkv, banded softmax(QK^T) V (half-width 8),
  out = attn @ Wproj + bproj.

Sharding: (batch x tokens) across the 8 cores -- each core owns a contiguous
block of 512 token rows. The attention band is 17 wide, so each core needs an
8-token halo of K/V context only: NO collectives.

v4 design:
  - inputs packed host-side into a few large DMAs spread over the
    sync/scalar/gpsimd issue queues (a dma_start costs ~600 ns of issue time
    on its queue engine); xh + the first qk weight block go first so real
    matmuls can start ~15 us in, behind a PE warmup chain that releases the
    HAM clock-gate (2.4 GHz) before real work arrives.
  - attention runs on 112-row query tiles whose k/v window is EXACTLY 128
    (112 + 2*8), so each (head, tile) needs ONE score matmul and ONE AV
    matmul -- no separate 16-row band remainder strip.
  - AV matmul runs in NATURAL layout (attention-weight strip stationary, V
    moving), so the softmax denominator lands per-partition: normalization
    is a native reciprocal [128,5] + tensor_scalar_mul. A [1,512]
    single-partition reciprocal would cost 4 us on DVE; this costs ~0.1 us.
  - normalized O tiles are transposed to the [dims, tokens] layout the
    output projection needs via XBAR DMA-transposes on the otherwise idle
    DMA engines, alternating sync/scalar issue queues.
  - output projection is c-outer across 8 PSUM banks (reusing every
    attention pool's bank), so only the last 8-matmul column depends on the
    final head; bias is folded in as a K=1 matmul against a ones row; the
    PSUM->SBUF copies and output DMAs alternate scalar/vector engines and
    scalar/sync queues.
"""

import sys

sys.path.insert(0, "/opt/trn_rl_repo")

import ml_dtypes
import numpy as np

import concourse.bass as bass
import concourse.tile as tile
from concourse import bacc, mybir
from concourse.bass_utils import run_bass_kernel_spmd

F32 = mybir.dt.float32
BF16 = mybir.dt.bfloat16
AF = mybir.ActivationFunctionType

B, N, C, H, HD, W = 2, 2048, 1024, 16, 64, 8
SCALE = float(HD) ** -0.5
CORES = 8
TOK = 512            # token rows owned per core
HALO = TOK + 2 * W   # 528 k/v context tokens per core
QT = 112             # query rows per attention tile (window = QT+2W = 128)
NWT = 5              # attention tiles per core: 4x112 + 1x64
NWARM = 48           # PE warmup matmuls (N=256 each)

# per-tile (query-rows, window-rows); last tile is the 64-row remainder
WTS = [(112, 128), (112, 128), (112, 128), (112, 128), (64, 80)]

_CACHE = {}


def _build_nc(dbg=False):
    nc = bacc.Bacc(None, target_bir_lowering=False)
    xhp = nc.dram_tensor("xhp", [128, 8 * HALO], BF16, kind="ExternalInput")
    wvp = nc.dram_tensor("wvp", [128, 8192], BF16, kind="ExternalInput")
    wqkp = nc.dram_tensor("wqkp", [128, 16384], BF16, kind="ExternalInput")
    wpp = nc.dram_tensor("wpp", [128, 8192], BF16, kind="ExternalInput")
    mA = nc.dram_tensor("mA", [128, TOK], BF16, kind="ExternalInput")
    bT = nc.dram_tensor("bT", [1, C], BF16, kind="ExternalInput")
    outT = nc.dram_tensor("outT", [C, TOK], F32, kind="ExternalOutput")

    with tile.TileContext(nc) as tc:
        with tc.tile_pool(name="persist", bufs=1) as pp:
            # ---- persistent SBUF ----
            wmA = pp.tile([128, 128], BF16, tag="wmA", name="wmA")
            wmB = pp.tile([128, 256], BF16, tag="wmB", name="wmB")
            ones_r = pp.tile([1, TOK], BF16, tag="ones_r", name="ones_r")
            xh = pp.tile([128, 8 * HALO], BF16, tag="xh", name="xh")
            wv_sb = pp.tile([128, 8192], BF16, tag="wv", name="wv")
            wqk_sb = pp.tile([128, 16384], BF16, tag="wqk", name="wqk")
            wp_sb = pp.tile([128, 8192], BF16, tag="wp", name="wp")
            mask_a = pp.tile([128, TOK], BF16, tag="mask_a", name="mask_a")
            biasT = pp.tile([1, C], BF16, tag="biasT", name="biasT")
            v1 = [pp.tile([pw, H, HD + 1], BF16, tag=f"v1_{t}", name=f"v1_{t}")
                  for t, (pq, pw) in enumerate(WTS)]
            # k-side qkT padded to 576 cols (zeros) so every score tile can
            # use a full 128-wide window matmul
            qkT = [pp.tile([128, 576 if m >= 8 else HALO], BF16,
                           tag=f"qkT{m}", name=f"qkT{m}") for m in range(16)]
            otn = [pp.tile([128, TOK], BF16, tag=f"otn{m}", name=f"otn{m}")
                   for m in range(8)]

            # warmup sources: first DVE work, no DMA dependency
            nc.vector.memset(wmA[:], 0.0)
            nc.vector.memset(wmB[:], 0.0)
            nc.vector.memset(ones_r[:], 1.0)
            for t in range(NWT):
                # ones column of v1 (disjoint from the v-proj copy columns)
                nc.vector.memset(v1[t][:, :, HD], 1.0)
            for m in range(8, 16):
                nc.vector.memset(qkT[m][:, HALO:576], 0.0)

            # ---- input DMAs: few, large, spread across issue queues ----
            # sync queue: the critical pair first (xh + first qk block)
            nc.sync.dma_start(out=xh[:], in_=xhp[:])
            nc.sync.dma_start(out=wqk_sb[:, 0:2048], in_=wqkp[:, 0:2048])
            nc.sync.dma_start(out=wv_sb[:, 0:4096], in_=wvp[:, 0:4096])
            nc.sync.dma_start(out=wv_sb[:, 4096:8192], in_=wvp[:, 4096:8192])
            nc.sync.dma_start(out=wp_sb[:], in_=wpp[:])
            # scalar queue: remaining qk weight blocks (m-major packing)
            for fm in range(1, 8):
                nc.scalar.dma_start(out=wqk_sb[:, 2048 * fm:2048 * (fm + 1)],
                                    in_=wqkp[:, 2048 * fm:2048 * (fm + 1)])
            # gpsimd (software DGE): small tensors
            nc.gpsimd.dma_start(out=mask_a[:], in_=mA[:])
            nc.gpsimd.dma_start(out=biasT[:], in_=bT[:])

            # PSUM budget (8 banks of 2 KiB):
            #   pa 1 + pk 1 + sb(pb) 1 + stA 1 + otb 2 + pv 2 = 8
            with tc.tile_pool(name="psQ", bufs=1, space="PSUM") as psQ, \
                 tc.tile_pool(name="psSB", bufs=1, space="PSUM") as psSB, \
                 tc.tile_pool(name="psSA", bufs=1, space="PSUM") as psSA, \
                 tc.tile_pool(name="psO", bufs=2, space="PSUM") as psO, \
                 tc.tile_pool(name="psV", bufs=2, space="PSUM") as psV, \
                 tc.tile_pool(name="atpa", bufs=2) as atpa, \
                 tc.tile_pool(name="recp", bufs=2) as recp, \
                 tc.tile_pool(name="onp", bufs=2) as onp, \
                 tc.tile_pool(name="outp", bufs=4) as outp:

                # ---- PE warmup: serialized dummy matmuls keep HAM busy
                # while the input DMAs stream ----
                psw = psV.tile([128, 512], F32, tag="pv", name="warm")
                for i in range(NWARM):
                    nc.tensor.matmul(psw[:, 0:256], wmA[:], wmB[:],
                                     start=True, stop=True)
                wsink = atpa.tile([128, TOK], BF16, tag="atA", name="wsink")
                nc.vector.tensor_copy(wsink[:, 0:256], psw[:, 0:256])

                ats = {}
                onats = {}

                def emit_qkproj(fm):
                    # q chunk m=fm (own tokens), k chunk m=8+fm (full halo)
                    pa = psQ.tile([128, 512], F32, tag="pa", name="pa")
                    for c in range(8):
                        nc.tensor.matmul(
                            pa[:],
                            wqk_sb[:, 2048 * fm + 128 * c:2048 * fm + 128 * (c + 1)],
                            xh[:, 528 * c + W:528 * c + W + TOK],
                            start=(c == 0), stop=(c == 7))
                    nc.vector.tensor_copy(qkT[fm][:, W:W + TOK], pa[:])
                    pk = psQ.tile([128, 512], F32, tag="pk", name="pk")
                    for c in range(8):
                        nc.tensor.matmul(
                            pk[:],
                            wqk_sb[:, 2048 * fm + 1024 + 128 * c:
                                   2048 * fm + 1024 + 128 * (c + 1)],
                            xh[:, 528 * c:528 * c + 512],
                            start=(c == 0), stop=(c == 7))
                    nc.scalar.copy(qkT[8 + fm][:, 0:512], pk[:])
                    sb = psSB.tile([128, 512], F32, tag="sb", name="pb")
                    for c in range(8):
                        nc.tensor.matmul(
                            sb[:, 0:2 * W],
                            wqk_sb[:, 2048 * fm + 1024 + 128 * c:
                                   2048 * fm + 1024 + 128 * (c + 1)],
                            xh[:, 528 * c + 512:528 * c + 528],
                            start=(c == 0), stop=(c == 7))
                    nc.scalar.copy(qkT[8 + fm][:, 512:528], sb[:, 0:2 * W])

                def emit_scores(fm, h):
                    # transposed score strips + exp + band mask, one head.
                    # tile i: window = halo[112i : 112i+128], queries
                    # 112i..112i+111 at strip cols 112i.. -- one matmul each.
                    off = (h % 2) * 64
                    stA = psSA.tile([128, TOK], F32, tag="stA", name="stA")
                    col = 0
                    for t, (pq, pw) in enumerate(WTS):
                        s = QT * t
                        q_ap = qkT[fm][off:off + 64, W + s:W + s + pq]
                        k1 = qkT[8 + fm][off:off + 64, s:s + 128]
                        nc.tensor.matmul(stA[:, col:col + pq], k1, q_ap,
                                         start=True, stop=True)
                        col += pq
                    atA = atpa.tile([128, TOK], BF16, tag="atA", name="atA")
                    nc.scalar.activation(atA[:], stA[:], AF.Exp)
                    nc.vector.tensor_mul(atA[:], atA[:], mask_a[:])
                    ats[h] = atA

                def emit_av(fm, h):
                    # natural-layout O strip per tile: [<=112 q, 65] where
                    # col 64 = softmax denominator (per-partition!)
                    off = (h % 2) * 64
                    atA = ats.pop(h)
                    onat = psO.tile([128, 512], F32, tag="otb", name="onat")
                    col = 0
                    for t, (pq, pw) in enumerate(WTS):
                        nc.tensor.matmul(onat[0:pq, 65 * t:65 * t + 65],
                                         atA[0:pw, col:col + pq],
                                         v1[t][:, h, :],
                                         start=True, stop=True)
                        col += pq
                    if fm not in onats:
                        onats[fm] = onp.tile([128, NWT, 128], BF16, tag="on",
                                             name="on")
                    on_nat = onats[fm]
                    rec = recp.tile([128, NWT], F32, tag="rec", name="rec")
                    nc.vector.reciprocal(
                        rec[:, 0:4],
                        onat[:, 0:260].rearrange("p (t x) -> p t x", x=65)
                        [:, :, 64])
                    nc.vector.reciprocal(rec[0:64, 4:5],
                                         onat[0:64, 4 * 65 + 64:4 * 65 + 65])
                    # normalize per tile, tile-local partition base 0
                    for t, (pq, pw) in enumerate(WTS):
                        nc.vector.tensor_scalar_mul(
                            on_nat[0:pq, t, off:off + 64],
                            onat[0:pq, 65 * t:65 * t + 64],
                            rec[0:pq, t:t + 1])

                def emit_transposes(fm):
                    # [q, dims] -> [dims, q] via XBAR DMA (idle DMA engines)
                    on_nat = onats.pop(fm)
                    col = 0
                    for t, (pq, pw) in enumerate(WTS):
                        eng = nc.sync if t % 2 == 0 else nc.scalar
                        eng.dma_start(out=otn[fm][:, col:col + pq],
                                      in_=on_nat[0:pq, t, :], transpose=True)
                        col += pq

                def emit_vproj():
                    # v = x @ Wv in natural [token, head, dim+1] layout at
                    # the 112-stride window offsets (windows overlap; the
                    # matmul count is unchanged). 65th column = 1.0 so AV
                    # also sums the denominators. half0 groups first.
                    for half in range(2):
                        for t, (pq, pw) in enumerate(WTS):
                            s = QT * t
                            pv = psV.tile([128, 512], F32, tag="pv",
                                          name=f"pv{half}_{t}")
                            for c in range(8):
                                nc.tensor.matmul(
                                    pv[:pw, :],
                                    xh[:, 528 * c + s:528 * c + s + pw],
                                    wv_sb[:, 4096 * half + 512 * c:
                                          4096 * half + 512 * c + 512],
                                    start=(c == 0), stop=(c == 7))
                            nc.vector.tensor_copy(
                                v1[t][:, 8 * half:8 * half + 8, 0:HD],
                                pv[:pw, :].rearrange("p (h d) -> p h d", d=HD))

                # ---- emission order (engine queues are FIFO) ----
                emit_qkproj(0)
                emit_scores(0, 0)
                emit_scores(0, 1)
                emit_qkproj(1)
                emit_vproj()
                emit_scores(1, 2)
                emit_av(0, 0)
                emit_scores(1, 3)
                emit_av(0, 1)
                emit_transposes(0)
                for fm in range(2, 8):
                    emit_qkproj(fm)
                    emit_av(fm - 1, 2 * fm - 2)
                    emit_scores(fm, 2 * fm)
                    emit_av(fm - 1, 2 * fm - 1)
                    emit_transposes(fm - 1)
                    emit_scores(fm, 2 * fm + 1)
                emit_av(7, 14)
                emit_av(7, 15)
                emit_transposes(7)

                # ---- output projection, c-outer over 8 PSUM banks ----
                pf = [psV.tile([128, 512], F32, tag="pv", name="pf0"),
                      psV.tile([128, 512], F32, tag="pv", name="pf1"),
                      psQ.tile([128, 512], F32, tag="pa", name="pf2"),
                      psQ.tile([128, 512], F32, tag="pk", name="pf3"),
                      psSA.tile([128, 512], F32, tag="stA", name="pf4"),
                      psSB.tile([128, 512], F32, tag="sb", name="pf5"),
                      psO.tile([128, 512], F32, tag="otb", name="pf6"),
                      psO.tile([128, 512], F32, tag="otb", name="pf7")]
                for c in range(8):
                    for m in range(8):
                        nc.tensor.matmul(
                            pf[m][:],
                            wp_sb[:, 1024 * c + 128 * m:1024 * c + 128 * (m + 1)],
                            otn[c][:],
                            start=(c == 0), stop=False)
                for m in range(8):
                    # bias folded in as a K=1 matmul against a ones row
                    nc.tensor.matmul(pf[m][:], biasT[0:1, 128 * m:128 * (m + 1)],
                                     ones_r[0:1, :], start=False, stop=True)
                for m in range(8):
                    ob = outp.tile([128, 512], F32, tag="ob", name="ob")
                    if m % 2 == 0:
                        nc.scalar.copy(ob[:], pf[m][:])
                        nc.scalar.dma_start(out=outT[128 * m:128 * (m + 1), :],
                                            in_=ob[:])
                    else:
                        nc.vector.tensor_copy(ob[:], pf[m][:])
                        nc.sync.dma_start(out=outT[128 * m:128 * (m + 1), :],
                                          in_=ob[:])

    nc.finalize()
    return nc


def _get_nc(dbg=False):
    key = ("nc", dbg)
    if key not in _CACHE:
        _CACHE[key] = _build_nc(dbg)
    return _CACHE[key]


def _band_mask_np(n, w):
    i = np.arange(n)[:, None]
    j = np.arange(n)[None, :]
    lo = np.where(i <= w, 0, i - w)
    hi = np.where(n - i <= w, n - 1, i + w)
    return (j >= lo) & (j <= hi)


def _make_in_maps(x, Wqkv, Wproj, bproj):
    x = np.ascontiguousarray(np.asarray(x, dtype=np.float32))
    Wqkv = np.asarray(Wqkv, dtype=np.float32)
    Wproj = np.ascontiguousarray(np.asarray(Wproj, dtype=np.float32))
    bproj = np.asarray(bproj, dtype=np.float32)

    wqk_host = np.concatenate(
        [Wqkv[:, :C] * np.float32(SCALE), Wqkv[:, C:2 * C]], axis=1)
    wqk_host = np.ascontiguousarray(wqk_host).astype(ml_dtypes.bfloat16)
    wv_host = np.ascontiguousarray(Wqkv[:, 2 * C:]).astype(ml_dtypes.bfloat16)
    wp_host = Wproj.astype(ml_dtypes.bfloat16)
    bT_host = np.ascontiguousarray(bproj.reshape(1, C)).astype(ml_dtypes.bfloat16)
    band = _band_mask_np(N, W)

    # packed weight layouts (shared by all cores)
    wqkp_host = np.concatenate(
        [np.concatenate(
            [wqk_host[128 * c:128 * (c + 1), 128 * fm:128 * (fm + 1)]
             for c in range(8)] +
            [wqk_host[128 * c:128 * (c + 1), 1024 + 128 * fm:1024 + 128 * (fm + 1)]
             for c in range(8)], axis=1)
         for fm in range(8)], axis=1)
    wqkp_host = np.ascontiguousarray(wqkp_host)
    wvp_host = np.concatenate(
        [np.concatenate([wv_host[128 * c:128 * (c + 1), 512 * half:512 * (half + 1)]
                         for c in range(8)], axis=1)
         for half in range(2)], axis=1)
    wvp_host = np.ascontiguousarray(wvp_host)
    wpp_host = np.concatenate(
        [wp_host[128 * c:128 * (c + 1), :] for c in range(8)], axis=1)
    wpp_host = np.ascontiguousarray(wpp_host)

    in_maps = []
    for core in range(CORES):
        b, qt = divmod(core, 4)
        g0 = qt * TOK
        xhrows = np.zeros((HALO, C), np.float32)
        s = max(0, g0 - W)
        e = min(N, g0 + TOK + W)
        xhrows[s - (g0 - W):e - (g0 - W)] = x[b, s:e]
        xhT_host = np.ascontiguousarray(xhrows.T).astype(ml_dtypes.bfloat16)
        xhp_host = np.ascontiguousarray(np.concatenate(
            [xhT_host[128 * c:128 * (c + 1), :] for c in range(8)], axis=1))

        # mask strip in 112-tile packing: col QT*t + r <-> query g0+QT*t+r,
        # row w <-> key (g0 - W) + QT*t + w
        mAh = np.zeros((128, TOK), np.float32)
        col = 0
        for t, (pq, pw) in enumerate(WTS):
            s0 = QT * t
            i = g0 + s0 + np.arange(pq)[None, :]
            jw = (g0 - W) + s0 + np.arange(pw)[:, None]
            valid = (jw >= 0) & (jw < N)
            mm = band[i, np.clip(jw, 0, N - 1)] & valid
            mAh[0:pw, col:col + pq] = mm
            col += pq
        in_maps.append({
            "xhp": xhp_host, "wvp": wvp_host, "wqkp": wqkp_host,
            "wpp": wpp_host, "bT": bT_host,
            "mA": mAh.astype(ml_dtypes.bfloat16),
        })
    return in_maps


def run_spmd(x, Wqkv, Wproj, bproj, dbg=False, **kw):
    """Run the SPMD kernel; returns (output, BassKernelResults)."""
    nc = _get_nc(dbg)
    in_maps = _make_in_maps(x, Wqkv, Wproj, bproj)
    res = run_bass_kernel_spmd(nc, in_maps, list(range(CORES)), **kw)
    outT = np.concatenate([res.results[i]["outT"] for i in range(CORES)], axis=1)
    out = np.ascontiguousarray(outT.T).reshape(B, N, C)
    return out, res


def kernel(x, Wqkv, Wproj, bproj):
    out, _ = run_spmd(x, Wqkv, Wproj, bproj)
    return out


# revision 30
# speedup vs baseline: 1.7832x; 1.0168x over previous
"""Banded (sparse) attention + projections on 8 Trainium2 NeuronCores.

Problem: nn_Attention_old_90211493085279
  x [2, 2048, 1024] -> qkv = x @ Wqkv, banded softmax(QK^T) V (half-width 8),
  out = attn @ Wproj + bproj.

Sharding: (batch x tokens) across the 8 cores -- each core owns a contiguous
block of 512 token rows. The attention band is 17 wide, so each core needs an
8-token halo of K/V context only: NO collectives.

v4 design:
  - inputs packed host-side into a few large DMAs spread over the
    sync/scalar/gpsimd issue queues (a dma_start costs ~600 ns of issue time
    on its queue engine); xh + the first qk weight block go first so real
    matmuls can start ~15 us in, behind a PE warmup chain that releases the
    HAM clock-gate (2.4 GHz) before real work arrives.
  - attention runs on 112-row query tiles whose k/v window is EXACTLY 128
    (112 + 2*8), so each (head, tile) needs ONE score matmul and ONE AV
    matmul -- no separate 16-row band remainder strip.
  - AV matmul runs in NATURAL layout (attention-weight strip stationary, V
    moving), so the softmax denominator lands per-partition: normalization
    is a native reciprocal [128,5] + tensor_scalar_mul. A [1,512]
    single-partition reciprocal would cost 4 us on DVE; this costs ~0.1 us.
  - normalized O tiles are transposed to the [dims, tokens] layout the
    output projection needs via XBAR DMA-transposes on the otherwise idle
    DMA engines, alternating sync/scalar issue queues.
  - output projection is c-outer across 8 PSUM banks (reusing every
    attention pool's bank), so only the last 8-matmul column depends on the
    final head; bias is folded in as a K=1 matmul against a ones row; the
    PSUM->SBUF copies and output DMAs alternate scalar/vector engines and
    scalar/sync queues.
"""

import sys

sys.path.insert(0, "/opt/trn_rl_repo")

import ml_dtypes
import numpy as np

import concourse.bass as bass
import concourse.tile as tile
from concourse import bacc, mybir
from concourse.bass_utils import run_bass_kernel_spmd

F32 = mybir.dt.float32
BF16 = mybir.dt.bfloat16
AF = mybir.ActivationFunctionType

B, N, C, H, HD, W = 2, 2048, 1024, 16, 64, 8
SCALE = float(HD) ** -0.5
CORES = 8
TOK = 512            # token rows owned per core
HALO = TOK + 2 * W   # 528 k/v context tokens per core
QT = 112             # query rows per attention tile (window = QT+2W = 128)
NWT = 5              # attention tiles per core: 4x112 + 1x64
NWARM = 48           # PE warmup matmuls (N=256 each)

# per-tile (query-rows, window-rows); last tile is the 64-row remainder
WTS = [(112, 128), (112, 128), (112, 128), (112, 128), (64, 80)]

_CACHE = {}


def _build_nc(dbg=False):
    nc = bacc.Bacc(None, target_bir_lowering=False)
    xhp = nc.dram_tensor("xhp", [128, 8 * HALO], BF16, kind="ExternalInput")
    wvp = nc.dram_tensor("wvp", [128, 8192], BF16, kind="ExternalInput")
    wqkp = nc.dram_tensor("wqkp", [128, 16384], BF16, kind="ExternalInput")
    wpp = nc.dram_tensor("wpp", [128, 8192], BF16, kind="ExternalInput")
    mA = nc.dram_tensor("mA", [128, TOK], BF16, kind="ExternalInput")
    bT = nc.dram_tensor("bT", [1, C], BF16, kind="ExternalInput")
    outT = nc.dram_tensor("outT", [C, TOK], BF16, kind="ExternalOutput")

    with tile.TileContext(nc) as tc:
        with tc.tile_pool(name="persist", bufs=1) as pp:
            # ---- persistent SBUF ----
            wmA = pp.tile([128, 128], BF16, tag="wmA", name="wmA")
            wmB = pp.tile([128, 256], BF16, tag="wmB", name="wmB")
            ones_r = pp.tile([1, TOK], BF16, tag="ones_r", name="ones_r")
            xh = pp.tile([128, 8 * HALO], BF16, tag="xh", name="xh")
            wv_sb = pp.tile([128, 8192], BF16, tag="wv", name="wv")
            wqk_sb = pp.tile([128, 16384], BF16, tag="wqk", name="wqk")
            wp_sb = pp.tile([128, 8192], BF16, tag="wp", name="wp")
            mask_a = pp.tile([128, TOK], BF16, tag="mask_a", name="mask_a")
            biasT = pp.tile([1, C], BF16, tag="biasT", name="biasT")
            v1 = [pp.tile([pw, H, HD + 1], BF16, tag=f"v1_{t}", name=f"v1_{t}")
                  for t, (pq, pw) in enumerate(WTS)]
            # k-side qkT padded to 576 cols (zeros) so every score tile can
            # use a full 128-wide window matmul
            qkT = [pp.tile([128, 576 if m >= 8 else HALO], BF16,
                           tag=f"qkT{m}", name=f"qkT{m}") for m in range(16)]
            otn = [pp.tile([128, TOK], BF16, tag=f"otn{m}", name=f"otn{m}")
                   for m in range(8)]

            # warmup sources: first DVE work, no DMA dependency
            nc.vector.memset(wmA[:], 0.0)
            nc.vector.memset(wmB[:], 0.0)
            nc.vector.memset(ones_r[:], 1.0)
            for t in range(NWT):
                # ones column of v1 (disjoint from the v-proj copy columns)
                nc.vector.memset(v1[t][:, :, HD], 1.0)
            for m in range(8, 16):
                nc.vector.memset(qkT[m][:, HALO:576], 0.0)

            # ---- input DMAs: few, large, spread across issue queues ----
            # sync queue: the critical pair first (xh + first qk block);
            # the sync queue's DGE starts ~3 us before the scalar queue's
            nc.sync.dma_start(out=xh[:], in_=xhp[:])
            nc.sync.dma_start(out=wqk_sb[:, 0:2048], in_=wqkp[:, 0:2048])
            nc.sync.dma_start(out=wv_sb[:, 0:4096], in_=wvp[:, 0:4096])
            nc.sync.dma_start(out=wv_sb[:, 4096:8192], in_=wvp[:, 4096:8192])
            nc.sync.dma_start(out=wp_sb[:], in_=wpp[:])
            # scalar queue: remaining qk weight blocks (m-major packing)
            for fm in range(1, 8):
                nc.scalar.dma_start(out=wqk_sb[:, 2048 * fm:2048 * (fm + 1)],
                                    in_=wqkp[:, 2048 * fm:2048 * (fm + 1)])
            # gpsimd (software DGE): small tensors
            nc.gpsimd.dma_start(out=mask_a[:], in_=mA[:])
            nc.gpsimd.dma_start(out=biasT[:], in_=bT[:])

            # PSUM budget (8 banks of 2 KiB):
            #   pa 1 + pk 1 + pb 1 + stA 1 + otb 2 + pv 2 = 8
            with tc.tile_pool(name="psQ", bufs=1, space="PSUM") as psQ, \
                 tc.tile_pool(name="psSB", bufs=1, space="PSUM") as psSB, \
                 tc.tile_pool(name="psSA", bufs=1, space="PSUM") as psSA, \
                 tc.tile_pool(name="psO", bufs=2, space="PSUM") as psO, \
                 tc.tile_pool(name="psV", bufs=2, space="PSUM") as psV, \
                 tc.tile_pool(name="atpa", bufs=2) as atpa, \
                 tc.tile_pool(name="recp", bufs=2) as recp, \
                 tc.tile_pool(name="bcp", bufs=2) as bcp, \
                 tc.tile_pool(name="outp", bufs=8) as outp:

                # ---- PE warmup: serialized dummy matmuls keep HAM busy
                # while the input DMAs stream ----
                psw = psV.tile([128, 512], F32, tag="pv", name="warm")
                for i in range(NWARM):
                    nc.tensor.matmul(psw[:, 0:256], wmA[:], wmB[:],
                                     start=True, stop=True)
                wsink = atpa.tile([128, TOK], BF16, tag="atA", name="wsink")
                nc.vector.tensor_copy(wsink[:, 0:256], psw[:, 0:256])

                ats = {}

                def emit_qkproj(fm):
                    # q chunk m=fm (own tokens), k chunk m=8+fm (full halo)
                    pa = psQ.tile([128, 512], F32, tag="pa", name="pa")
                    for c in range(8):
                        nc.tensor.matmul(
                            pa[:],
                            wqk_sb[:, 2048 * fm + 128 * c:2048 * fm + 128 * (c + 1)],
                            xh[:, 528 * c + W:528 * c + W + TOK],
                            start=(c == 0), stop=(c == 7))
                    nc.vector.tensor_copy(qkT[fm][:, W:W + TOK], pa[:])
                    pk = psQ.tile([128, 512], F32, tag="pk", name="pk")
                    for c in range(8):
                        nc.tensor.matmul(
                            pk[:],
                            wqk_sb[:, 2048 * fm + 1024 + 128 * c:
                                   2048 * fm + 1024 + 128 * (c + 1)],
                            xh[:, 528 * c:528 * c + 512],
                            start=(c == 0), stop=(c == 7))
                    nc.scalar.copy(qkT[8 + fm][:, 0:512], pk[:])
                    sb = psSB.tile([128, 512], F32, tag="sb", name="pb")
                    for c in range(8):
                        nc.tensor.matmul(
                            sb[:, 0:2 * W],
                            wqk_sb[:, 2048 * fm + 1024 + 128 * c:
                                   2048 * fm + 1024 + 128 * (c + 1)],
                            xh[:, 528 * c + 512:528 * c + 528],
                            start=(c == 0), stop=(c == 7))
                    nc.scalar.copy(qkT[8 + fm][:, 512:528], sb[:, 0:2 * W])

                def emit_scores(fm, h):
                    # transposed score strips + exp + band mask, one head.
                    # tile i: window = halo[112i : 112i+128], queries
                    # 112i..112i+111 at strip cols 112i.. -- one matmul each.
                    off = (h % 2) * 64
                    stA = psSA.tile([128, TOK], F32, tag="stA", name="stA")
                    col = 0
                    for t, (pq, pw) in enumerate(WTS):
                        s = QT * t
                        q_ap = qkT[fm][off:off + 64, W + s:W + s + pq]
                        k1 = qkT[8 + fm][off:off + 64, s:s + 128]
                        nc.tensor.matmul(stA[:, col:col + pq], k1, q_ap,
                                         start=True, stop=True)
                        col += pq
                    atA = atpa.tile([128, TOK], BF16, tag="atA", name="atA")
                    nc.scalar.activation(atA[:], stA[:], AF.Exp)
                    nc.vector.tensor_mul(atA[:], atA[:], mask_a[:])
                    ats[h] = atA

                def emit_av(fm, h):
                    # transposed O strip [65, 512] per head (V stationary,
                    # attention weights moving); row 64 = softmax denominator.
                    # Normalization chain never touches the PE: DVE
                    # approx-reciprocal [1,512] -> GpSimd partition broadcast
                    # -> fused DVE multiply into otn.
                    off = (h % 2) * 64
                    atA = ats.pop(h)
                    otb = psO.tile([128, 512], F32, tag="otb", name="otb")
                    col = 0
                    for t, (pq, pw) in enumerate(WTS):
                        nc.tensor.matmul(otb[0:HD + 1, col:col + pq],
                                         v1[t][:, h, :],
                                         atA[0:pw, col:col + pq],
                                         start=True, stop=True)
                        col += pq
                    den = recp.tile([1, TOK], F32, tag="den", name="den")
                    nc.scalar.copy(den[:], otb[HD:HD + 1, :])
                    rec = recp.tile([1, TOK], F32, tag="rec", name="rec")
                    nc.vector.reciprocal_approx_fast(rec[:], den[:])
                    bc = bcp.tile([HD, TOK], F32, tag="bc", name="bc")
                    nc.gpsimd.partition_broadcast(bc[:], rec[0:1, :])
                    nc.vector.tensor_mul(otn[fm][off:off + 64, :],
                                         otb[0:HD, :], bc[:])

                def emit_vproj():
                    # v = x @ Wv in natural [token, head, dim+1] layout at
                    # the 112-stride window offsets (windows overlap; the
                    # matmul count is unchanged). 65th column = 1.0 so AV
                    # also sums the denominators. half0 groups first.
                    for half in range(2):
                        for t, (pq, pw) in enumerate(WTS):
                            s = QT * t
                            pv = psV.tile([128, 512], F32, tag="pv",
                                          name=f"pv{half}_{t}")
                            for c in range(8):
                                nc.tensor.matmul(
                                    pv[:pw, :],
                                    xh[:, 528 * c + s:528 * c + s + pw],
                                    wv_sb[:, 4096 * half + 512 * c:
                                          4096 * half + 512 * c + 512],
                                    start=(c == 0), stop=(c == 7))
                            nc.vector.tensor_copy(
                                v1[t][:, 8 * half:8 * half + 8, 0:HD],
                                pv[:pw, :].rearrange("p (h d) -> p h d", d=HD))

                # ---- emission order (engine queues are FIFO) ----
                emit_qkproj(0)
                emit_scores(0, 0)
                emit_scores(0, 1)
                emit_qkproj(1)
                emit_vproj()
                emit_scores(1, 2)
                emit_av(0, 0)
                emit_scores(1, 3)
                emit_av(0, 1)
                for fm in range(2, 8):
                    emit_qkproj(fm)
                    emit_av(fm - 1, 2 * fm - 2)
                    emit_scores(fm, 2 * fm)
                    emit_av(fm - 1, 2 * fm - 1)
                    emit_scores(fm, 2 * fm + 1)
                emit_av(7, 14)
                emit_av(7, 15)

                # ---- output projection, c-outer over 8 PSUM banks ----
                pf = [psV.tile([128, 512], F32, tag="pv", name="pf0"),
                      psV.tile([128, 512], F32, tag="pv", name="pf1"),
                      psQ.tile([128, 512], F32, tag="pa", name="pf2"),
                      psQ.tile([128, 512], F32, tag="pk", name="pf3"),
                      psSA.tile([128, 512], F32, tag="stA", name="pf4"),
                      psSB.tile([128, 512], F32, tag="sb", name="pf5"),
                      psO.tile([128, 512], F32, tag="otb", name="pf6"),
                      psO.tile([128, 512], F32, tag="otb", name="pf7")]
                for c in range(8):
                    for m in range(8):
                        nc.tensor.matmul(
                            pf[m][:],
                            wp_sb[:, 1024 * c + 128 * m:1024 * c + 128 * (m + 1)],
                            otn[c][:],
                            start=(c == 0), stop=False)
                for m in range(8):
                    # bias folded in as a K=1 matmul closing each group
                    nc.tensor.matmul(pf[m][:], biasT[0:1, 128 * m:128 * (m + 1)],
                                     ones_r[0:1, :], start=False, stop=True)
                for m in range(8):
                    ob = outp.tile([128, 512], BF16, tag="ob", name="ob")
                    if m % 2 == 0:
                        nc.scalar.copy(ob[:], pf[m][:])
                        nc.scalar.dma_start(out=outT[128 * m:128 * (m + 1), :],
                                            in_=ob[:])
                    else:
                        nc.vector.tensor_copy(ob[:], pf[m][:])
                        nc.sync.dma_start(out=outT[128 * m:128 * (m + 1), :],
                                          in_=ob[:])

    nc.finalize()
    return nc


def _get_nc(dbg=False):
    key = ("nc", dbg)
    if key not in _CACHE:
        _CACHE[key] = _build_nc(dbg)
    return _CACHE[key]


def _band_mask_np(n, w):
    i = np.arange(n)[:, None]
    j = np.arange(n)[None, :]
    lo = np.where(i <= w, 0, i - w)
    hi = np.where(n - i <= w, n - 1, i + w)
    return (j >= lo) & (j <= hi)


def _make_in_maps(x, Wqkv, Wproj, bproj):
    x = np.ascontiguousarray(np.asarray(x, dtype=np.float32))
    Wqkv = np.asarray(Wqkv, dtype=np.float32)
    Wproj = np.ascontiguousarray(np.asarray(Wproj, dtype=np.float32))
    bproj = np.asarray(bproj, dtype=np.float32)

    wqk_host = np.concatenate(
        [Wqkv[:, :C] * np.float32(SCALE), Wqkv[:, C:2 * C]], axis=1)
    wqk_host = np.ascontiguousarray(wqk_host).astype(ml_dtypes.bfloat16)
    wv_host = np.ascontiguousarray(Wqkv[:, 2 * C:]).astype(ml_dtypes.bfloat16)
    wp_host = Wproj.astype(ml_dtypes.bfloat16)
    bT_host = np.ascontiguousarray(bproj.reshape(1, C)).astype(ml_dtypes.bfloat16)
    band = _band_mask_np(N, W)

    # packed weight layouts (shared by all cores)
    wqkp_host = np.concatenate(
        [np.concatenate(
            [wqk_host[128 * c:128 * (c + 1), 128 * fm:128 * (fm + 1)]
             for c in range(8)] +
            [wqk_host[128 * c:128 * (c + 1), 1024 + 128 * fm:1024 + 128 * (fm + 1)]
             for c in range(8)], axis=1)
         for fm in range(8)], axis=1)
    wqkp_host = np.ascontiguousarray(wqkp_host)
    wvp_host = np.concatenate(
        [np.concatenate([wv_host[128 * c:128 * (c + 1), 512 * half:512 * (half + 1)]
                         for c in range(8)], axis=1)
         for half in range(2)], axis=1)
    wvp_host = np.ascontiguousarray(wvp_host)
    wpp_host = np.concatenate(
        [wp_host[128 * c:128 * (c + 1), :] for c in range(8)], axis=1)
    wpp_host = np.ascontiguousarray(wpp_host)

    in_maps = []
    for core in range(CORES):
        b, qt = divmod(core, 4)
        g0 = qt * TOK
        xhrows = np.zeros((HALO, C), np.float32)
        s = max(0, g0 - W)
        e = min(N, g0 + TOK + W)
        xhrows[s - (g0 - W):e - (g0 - W)] = x[b, s:e]
        xhT_host = np.ascontiguousarray(xhrows.T).astype(ml_dtypes.bfloat16)
        xhp_host = np.ascontiguousarray(np.concatenate(
            [xhT_host[128 * c:128 * (c + 1), :] for c in range(8)], axis=1))

        # mask strip in 112-tile packing: col QT*t + r <-> query g0+QT*t+r,
        # row w <-> key (g0 - W) + QT*t + w
        mAh = np.zeros((128, TOK), np.float32)
        col = 0
        for t, (pq, pw) in enumerate(WTS):
            s0 = QT * t
            i = g0 + s0 + np.arange(pq)[None, :]
            jw = (g0 - W) + s0 + np.arange(pw)[:, None]
            valid = (jw >= 0) & (jw < N)
            mm = band[i, np.clip(jw, 0, N - 1)] & valid
            mAh[0:pw, col:col + pq] = mm
            col += pq
        in_maps.append({
            "xhp": xhp_host, "wvp": wvp_host, "wqkp": wqkp_host,
            "wpp": wpp_host, "bT": bT_host,
            "mA": mAh.astype(ml_dtypes.bfloat16),
        })
    return in_maps


def run_spmd(x, Wqkv, Wproj, bproj, dbg=False, **kw):
    """Run the SPMD kernel; returns (output, BassKernelResults)."""
    nc = _get_nc(dbg)
    in_maps = _make_in_maps(x, Wqkv, Wproj, bproj)
    res = run_bass_kernel_spmd(nc, in_maps, list(range(CORES)), **kw)
    outT = np.concatenate(
        [np.asarray(res.results[i]["outT"], dtype=np.float32)
         for i in range(CORES)], axis=1)
    out = np.ascontiguousarray(outT.T).reshape(B, N, C)
    return out, res


def kernel(x, Wqkv, Wproj, bproj):
    out, _ = run_spmd(x, Wqkv, Wproj, bproj)
    return out
